# revision 36
# baseline (speedup 1.0000x reference)
"""nn_Block_8512625181077: hybrid window-attention + Mamba (VMamba) block.

Bass/Trainium2 device kernel, data-parallel over batch across 8 NeuronCores
(2 images per core).  Layout: channels on SBUF partitions, spatial L=3136 on
the free dimension.  The selective scan runs as 64 tensor_tensor_scan
instructions per image (4 directions x 16 states) in bf16 with fp32 state;
decay factors exp(A[:,n]*delta) come off the scalar engine straight from
delta via per-partition scale APs, B/C row-broadcasts ride the DMA engines
via DRAM staging, and the sum over states accumulates in PSUM via
identity-matmul on the tensor engine.  Cross-scan/merge orderings are pure
access-pattern views (transpose/reverse folded into reads/writes).
"""

import os
import sys
import numpy as np

sys.path.insert(0, "/opt/trn_rl_repo")

# debug phase limiter: full | norm | attn | pre | scan | se
KPHASE = os.environ.get("KPHASE", "full")
KWIN = os.environ.get("KWIN", "all")

B, DIM, H, W = 16, 128, 56, 56
HD = DIM // 2            # 64
L = H * W                # 3136
WS = 7
NH = 8
HEAD = HD // NH          # 8
DST = 16
DI = 2 * HD              # 128
DTR = 4
K = 4
EPS = 1e-5
N_CORES = 8
B_SH = B // N_CORES      # 2 images per core
WT = WS * WS             # 49 tokens per window
RCH = 8                  # spatial row-chunk: 8 rows of 56
NCH = H // RCH           # 7 chunks
CW = RCH * W             # 448
NBC = DTR + 2 * DST      # 36

_INPUT_SPECS = [
    ("x", (B_SH, DIM, H, W)), ("wq", (HD, HD)), ("wk", (HD, HD)),
    ("wv", (HD, HD)), ("w_proj", (HD, HD)), ("b_proj", (HD,)),
    ("g_q", (HD,)), ("g_k", (HD,)), ("g_v", (HD,)), ("g_vm", (HD,)),
    ("g_mlp", (DIM,)), ("m_in_w", (DI, HD)), ("m_conv_w", (DI, 1, 3, 3)),
    ("m_conv_b", (DI,)), ("m_xproj", (K, DI, NBC)),
    ("m_dtw", (K, DI, DTR)), ("m_dtb", (K, DI)), ("m_Alog", (K * DI, DST)),
    ("m_D", (K * DI,)), ("m_out_w", (HD, DI)),
    ("se_w1", (DIM // 4, DIM)), ("se_b1", (DIM // 4,)),
    ("se_w2", (DIM, DIM // 4)), ("se_b2", (DIM,)),
    ("mlp_w1", (4 * DIM, DIM)), ("mlp_b1", (4 * DIM,)),
    ("mlp_w2", (DIM, 4 * DIM)), ("mlp_b2", (DIM,)),
    ("gamma1", (DIM,)), ("gamma2", (DIM,)),
]


def kv(ap, k):
    """View of a (P, 3136) spatial AP enumerated in scan order k (involution)."""
    if k == 0:
        return ap
    if k == 1:
        return ap.rearrange("p (h w) -> p w h", h=H)
    if k == 2:
        return ap[:, ::-1]
    return ap.rearrange("p (h w) -> p w h", h=H)[:, ::-1, ::-1]


def m2(flat_ap, k):
    """Reshape a contiguous (P, 3136) AP to pair with kv(., k) shapes."""
    if k in (1, 3):
        return flat_ap.rearrange("p (a b) -> p a b", a=W)
    return flat_ap


def emit(tc, out_x, ins):
    import concourse.bass as bass
    from concourse import mybir
    from contextlib import ExitStack
    FP32 = mybir.dt.float32
    BF16 = mybir.dt.bfloat16
    AF = mybir.ActivationFunctionType
    OP = mybir.AluOpType
    AX = mybir.AxisListType
    nc = tc.nc

    def brow(row_ap, p):
        """DRAM row (n,) -> (p, n) partition-broadcast AP."""
        return bass.AP(tensor=row_ap.tensor, offset=row_ap.offset,
                       ap=[[0, p]] + [list(d) for d in row_ap.ap])

    ctx = ExitStack()
    with ctx:
        wpool = ctx.enter_context(tc.tile_pool(name="weights", bufs=1))
        epsc = wpool.tile([DIM, 1], FP32, tag='epsc')
        nc.vector.memset(epsc, EPS)
        zc = wpool.tile([DIM, 1], FP32, tag='zc')
        nc.vector.memset(zc, 0.0)

        _wtag = [0]

        def wtile(shape, dt):
            _wtag[0] += 1
            return wpool.tile(list(shape), dt, tag=f"w{_wtag[0]}",
                              name=f"w{_wtag[0]}")

        def load_f32(shape, ap):
            t = wtile(shape, FP32)
            nc.sync.dma_start(t, ap)
            return t

        def to_bf16(src, scale_col=None):
            d = wtile(src.shape, BF16)
            if scale_col is not None:
                nc.vector.tensor_scalar_mul(d, src, scale_col)
            else:
                nc.vector.tensor_copy(d, src)
            return d

        def colvec(name, n):
            t = wtile([n, 1], FP32)
            nc.sync.dma_start(t, ins[name].rearrange("(a b) -> a b", b=1))
            return t

        tr = lambda ap: ap.rearrange("a b -> b a")

        g_q = colvec("g_q", HD); g_k = colvec("g_k", HD); g_v = colvec("g_v", HD)
        g_vm = wpool.tile([DIM, 1], FP32, tag='g_vm')
        nc.sync.dma_start(g_vm[HD:DIM],
                          ins["g_vm"].rearrange("(a b) -> a b", b=1))
        g_mlp = colvec("g_mlp", DIM)
        gamma1 = colvec("gamma1", DIM); gamma2 = colvec("gamma2", DIM)
        b_proj = colvec("b_proj", HD); m_conv_b = colvec("m_conv_b", DI)
        se_b1 = colvec("se_b1", DIM // 4); se_b2 = colvec("se_b2", DIM)
        mlp_b2 = colvec("mlp_b2", DIM)
        mlp_b1 = load_f32((DIM, 4), ins["mlp_b1"].rearrange("(a b) -> b a", b=DIM))
        mlp_b1s = wtile((DIM, 4), FP32)
        nc.vector.tensor_scalar_mul(mlp_b1s, mlp_b1, 1.702)

        sc = float(HEAD ** -0.5)
        wq_f = load_f32((HD, HD), tr(ins["wq"]))
        wqT = wpool.tile([HD, HD], BF16, tag='wqT')
        nc.vector.tensor_scalar(wqT, wq_f, g_q, sc, op0=OP.mult, op1=OP.mult)
        wkT = to_bf16(load_f32((HD, HD), tr(ins["wk"])), scale_col=g_k)
        wvT = to_bf16(load_f32((HD, HD), tr(ins["wv"])), scale_col=g_v)
        wpT = to_bf16(load_f32((HD, HD), tr(ins["w_proj"])))
        # head band mask: maskmat[c, h] = 1 iff c // 8 == h
        mi = wpool.tile([HD, NH], mybir.dt.int32, tag="mi")
        nc.gpsimd.iota(mi, pattern=[[-HEAD, NH]], base=0, channel_multiplier=1)
        m_ge = wpool.tile([HD, NH], BF16, tag="m_ge")
        m_le = wpool.tile([HD, NH], BF16, tag="m_le")
        nc.vector.tensor_scalar(m_ge, mi, 0, 1.0, op0=OP.is_ge, op1=OP.mult)
        nc.vector.tensor_scalar(m_le, mi, HEAD - 1, 1.0, op0=OP.is_le,
                                op1=OP.mult)
        maskmat = wpool.tile([HD, NH], BF16, tag="maskmat")
        nc.vector.tensor_tensor(maskmat, m_ge, m_le, op=OP.mult)

        m_in_f = wpool.tile([DIM, DI], FP32, tag='m_in_f')
        nc.sync.dma_start(m_in_f[HD:DIM], tr(ins["m_in_w"]))
        m_in_wT = wpool.tile([DIM, DI], BF16, tag='m_in_wT')
        nc.vector.tensor_scalar_mul(m_in_wT[HD:DIM], m_in_f[HD:DIM],
                                    g_vm[HD:DIM])
        m_out_wT = to_bf16(load_f32((DI, HD), tr(ins["m_out_w"])))
        xprojT = to_bf16(load_f32((DI, K * NBC),
                                  ins["m_xproj"].rearrange("k d c -> d k c")))
        # padded out-channel layout: dts at 0:4, B at 32:48, C at 64:80
        xprojP = wpool.tile([DI, K, 96], BF16, tag="xprojP")
        nc.vector.memset(xprojP, 0.0)
        for k in range(K):
            nc.vector.tensor_copy(xprojP[:, k, 0:DTR],
                                  xprojT[:, k * NBC:k * NBC + DTR])
            nc.vector.tensor_copy(xprojP[:, k, 32:32 + DST],
                                  xprojT[:, k * NBC + DTR:k * NBC + DTR + DST])
            nc.vector.tensor_copy(xprojP[:, k, 64:64 + DST],
                                  xprojT[:, k * NBC + DTR + DST:(k + 1) * NBC])
        dtwT = to_bf16(load_f32((DTR, K * DI),
                                ins["m_dtw"].rearrange("k d r -> r k d")))
        dtb = load_f32((DI, K), tr(ins["m_dtb"]))
        convw = load_f32((DI, 9),
                         ins["m_conv_w"].rearrange("d o a b -> d (o a b)"))
        alog = load_f32((DI, K * DST),
                        ins["m_Alog"].rearrange("(k d) n -> d k n", k=K))
        Aneg = wpool.tile([DI, K * DST], FP32, tag='Aneg')
        nc.scalar.activation(Aneg, alog, AF.Exp, bias=zc)
        nc.vector.tensor_scalar_mul(Aneg, Aneg, -1.0)
        m_D = load_f32((DI, K), ins["m_D"].rearrange("(k d) -> d k", k=K))
        Dsum = wpool.tile([DI, 1], FP32, tag='Dsum')
        nc.vector.reduce_sum(Dsum, m_D, AX.X)

        se_w1T = to_bf16(load_f32((DIM, DIM // 4), tr(ins["se_w1"])))
        se_w2T = to_bf16(load_f32((DIM // 4, DIM), tr(ins["se_w2"])))
        mlp_w1T = to_bf16(load_f32((DIM, 4 * DIM), tr(ins["mlp_w1"])),
                          scale_col=g_mlp)
        w2_f = wtile((DIM, 4, DIM), FP32)
        w2_src = ins["mlp_w2"].rearrange("c (j p) -> p j c", j=4)
        for j in range(4):
            nc.sync.dma_start(w2_f[:, j], w2_src[:, j])
        mlp_w2T = to_bf16(w2_f)

        onesrow = wpool.tile([1, DIM], FP32, tag='onesrow')      # fp32 ones row (broadcasts)
        onescolf = wpool.tile([DIM, 1], FP32, tag='onescolf')
        nc.vector.memset(onescolf, 1.0)
        nc.vector.memset(onesrow, 1.0)
        onescol = wpool.tile([DIM, 1], BF16, tag='onescol')      # bf16 ones col (reductions)
        nc.vector.memset(onescol, 1.0)
        onesc49 = wpool.tile([WT, 1], BF16, tag='onesc49')
        nc.vector.memset(onesc49, 1.0)
        it_i = wpool.tile([DIM, DIM], mybir.dt.int32, tag='it_i')
        ip_i = wpool.tile([DIM, 1], mybir.dt.int32, tag='ip_i')
        nc.gpsimd.iota(it_i, pattern=[[1, DIM]], base=0, channel_multiplier=0)
        nc.gpsimd.iota(ip_i, pattern=[[0, 1]], base=0, channel_multiplier=1)
        ident = wpool.tile([DIM, DIM], BF16, tag='ident')
        nc.vector.tensor_tensor(ident, it_i, ip_i.to_broadcast((DIM, DIM)),
                                op=OP.is_equal)

        dbg = nc.dram_tensor("dbg", [DIM, 4096], mybir.dt.float32).ap()
        stageB = nc.dram_tensor("stageB", [B_SH, K, DST, L], BF16).ap()
        stageC = nc.dram_tensor("stageC", [B_SH, K, DST, L], BF16).ap()

        xdram = ins["x"].rearrange("b c h w -> b c (h w)")
        odram = out_x.rearrange("b c h w -> b c (h w)")

        def bail(img, X):
            for c in range(NCH):
                cs = slice(c * CW, (c + 1) * CW)
                nc.sync.dma_start(odram[img][:, cs], X[:, cs])

        for img in range(B_SH):
            ictx = ExitStack()
            with ictx:
                gp = ictx.enter_context(tc.tile_pool(name=f"img{img}", bufs=1))
                rvp = ictx.enter_context(tc.tile_pool(name=f"rv{img}", bufs=2))
                X = gp.tile([DIM, L], FP32, tag="X")
                nc.sync.dma_start(X, xdram[img])

                # ---- input norms: xn = rms(x1)*?, xm = rms(x2) (gains folded)
                xsq = gp.tile([DIM, L], BF16, tag="xsq")
                nc.scalar.activation(xsq, X, AF.Square, bias=zc)
                xn = gp.tile([HD, L], BF16, tag="xn")
                xm = gp.tile([DIM, L], BF16, tag="xm")
                n1 = ExitStack()
                with n1:
                    pn = n1.enter_context(
                        tc.tile_pool(name=f"pn{img}", bufs=2, space="PSUM"))
                    for half, dst in ((0, xn), (1, xm)):
                        hs = slice(half * HD, (half + 1) * HD)
                        for c in range(NCH):
                            cs = slice(c * CW, (c + 1) * CW)
                            ps = pn.tile([1, CW], FP32, tag="rsum")
                            nc.tensor.matmul(ps, onescol[hs], xsq[hs, cs],
                                             start=True, stop=True)
                            rv = rvp.tile([1, CW], FP32, tag="rv")
                            nc.scalar.activation(rv, ps, AF.Sqrt,
                                                 bias=epsc[0:1],
                                                 scale=1.0 / HD)
                            nc.vector.reciprocal(rv, rv)
                            bc = pn.tile([DIM, CW], FP32, tag="bc")
                            nc.tensor.matmul(bc[hs], onesrow[0:1, 0:HD], rv,
                                             start=True, stop=True)
                            dv = dst[:, cs] if half == 0 else dst[hs, cs]
                            nc.vector.tensor_mul(dv, X[hs, cs], bc[hs])

                if KPHASE == "norm":
                    bail(img, X)
                    continue

                # ---------------- attention branch ----------------
                q = gp.tile([HD, L], BF16, tag="q0")
                kk = gp.tile([HD, L], BF16, tag="k0")
                v = gp.tile([HD, L], BF16, tag="v")

                # window-major write view for one 7-row band wy:
                # psum (P, 392) iterates (ty, wx, tx); dest flat layout is
                # (wy, wx, ty*7+tx)*49
                def wmband(t, wy, p0, p1):
                    wmv = t.rearrange("p (wy wx s) -> p wy wx s", wx=8, s=WT)
                    return wmv[p0:p1, wy].rearrange(
                        "p wx (ty tx) -> p ty wx tx", ty=WS)

                ACW = WS * W   # 392
                pqkv = ExitStack()
                with pqkv:
                    pp = pqkv.enter_context(
                        tc.tile_pool(name=f"pqkv{img}", bufs=2, space="PSUM"))
                    for c in range(8):
                        cs = slice(c * ACW, (c + 1) * ACW)
                        for dst, wT in ((q, wqT), (kk, wkT), (v, wvT)):
                            ps = pp.tile([HD, ACW], FP32, tag="mm")
                            nc.tensor.matmul(ps, wT, xn[:, cs],
                                             start=True, stop=True)
                            nc.vector.tensor_copy(
                                wmband(dst, c, 0, HD),
                                ps.rearrange("p (ty x) -> p ty x", ty=WS))

                oc = gp.tile([HD, L], BF16, tag="oc")
                apool = ictx.enter_context(tc.tile_pool(name=f"att{img}", bufs=2))
                wctx = ExitStack()
                with wctx:
                    pS = wctx.enter_context(
                        tc.tile_pool(name=f"pS{img}", bufs=2, space="PSUM"))
                    pSm = wctx.enter_context(
                        tc.tile_pool(name=f"pSm{img}", bufs=1, space="PSUM"))
                    for wy in range(H // WS):
                        for wx in range(W // WS):
                            def win(t):
                                return t.rearrange(
                                    "p (wy wx s) -> p wy wx s",
                                    wx=8, s=WT)[:, wy, wx]
                            if KWIN == "qkv":
                                continue
                            v_w = win(v)
                            q_w, k_w = win(q), win(kk)
                            kmsk = apool.tile([HD, NH, WT], BF16, tag="kmsk")
                            nc.vector.tensor_tensor(
                                kmsk,
                                k_w.unsqueeze(1).to_broadcast((HD, NH, WT)),
                                maskmat.unsqueeze(2).to_broadcast(
                                    (HD, NH, WT)),
                                op=OP.mult)
                            sT = pS.tile([WT, 512], FP32, tag="sT")
                            for h in range(NH):
                                nc.tensor.matmul(sT[:, h * 64:h * 64 + WT],
                                                 kmsk[:, h], q_w,
                                                 start=True, stop=True)
                            if KWIN == "sT":
                                sTc = apool.tile([WT, 512], FP32, tag="sTc")
                                nc.vector.tensor_copy(sTc, sT)
                                nc.sync.dma_start(dbg[0:WT, 0:512], sTc)
                                continue
                            eT = apool.tile([WT, NH, 64], BF16, tag="eT")
                            nc.scalar.activation(
                                eT[:, :, 0:WT],
                                sT.rearrange("p (h s) -> p h s", h=NH)[:, :, 0:WT],
                                AF.Exp, bias=zc[0:WT])
                            if KWIN == "eT":
                                nc.sync.dma_start(
                                    dbg[0:WT, 0:512].bitcast(BF16), eT)
                                continue
                            vT_ps = pSm.tile([WT, HD], BF16, tag="vT")
                            nc.tensor.matmul(
                                vT_ps, v_w, ident[0:HD, 0:HD],
                                is_transpose=True, start=True, stop=True)
                            vT = apool.tile([WT, HD], BF16, tag="vTs")
                            nc.vector.tensor_copy(vT, vT_ps)
                            if KWIN == "vT":
                                nc.sync.dma_start(
                                    dbg[0:WT, 0:HD].bitcast(BF16), vT)
                                continue
                            S_ps = pSm.tile([WT, NH], FP32, tag="S")
                            o_ps = pSm.tile([WT, HD], FP32, tag="o")
                            for h in range(NH):
                                hs = slice(h * HEAD, (h + 1) * HEAD)
                                nc.tensor.matmul(S_ps[:, h:h + 1],
                                                 eT[:, h, 0:WT], onesc49,
                                                 start=True, stop=True)
                                nc.tensor.matmul(o_ps[:, hs], eT[:, h, 0:WT],
                                                 vT[:, hs],
                                                 start=True, stop=True)
                            Sinv = apool.tile([WT, NH], FP32, tag="Sinv")
                            nc.vector.reciprocal(Sinv, S_ps)
                            o_div = apool.tile([WT, NH, HEAD], BF16, tag="odiv")
                            nc.vector.tensor_mul(
                                o_div,
                                o_ps.rearrange("p (h d) -> p h d", h=NH),
                                Sinv.unsqueeze(2).to_broadcast(
                                    (WT, NH, HEAD)))
                            if KWIN == "odiv":
                                nc.sync.dma_start(
                                    dbg[0:WT, 0:HD].bitcast(BF16),
                                    o_div.rearrange("p h d -> p (h d)"))
                                continue
                            ocT_ps = pSm.tile([HD, WT], BF16, tag="ocT")
                            nc.tensor.matmul(
                                ocT_ps,
                                o_div.rearrange("p h d -> p (h d)"),
                                ident[0:WT, 0:WT], is_transpose=True,
                                start=True, stop=True)
                            nc.vector.tensor_copy(win(oc), ocT_ps)

                if KWIN != "all":
                    bail(img, X)
                    continue
                yawm = gp.tile([HD, L], BF16, tag="k0")    # reuse kk slot
                ya = gp.tile([HD, L], BF16, tag="q0")      # spatial
                ya_part = gp.tile([HD, 8], FP32, tag="yap")
                yav = ya.rearrange("p (h w) -> p h w", h=H)
                pproj = ExitStack()
                with pproj:
                    pp = pproj.enter_context(
                        tc.tile_pool(name=f"ppr{img}", bufs=2, space="PSUM"))
                    for c in range(8):   # window-row bands
                        cs = slice(c * ACW, (c + 1) * ACW)
                        ps = pp.tile([HD, ACW], FP32, tag="mm")
                        nc.tensor.matmul(ps, wpT, oc[:, cs], start=True,
                                         stop=True)
                        nc.vector.scalar_tensor_tensor(
                            yawm[:, cs], ps, 1.0,
                            b_proj.to_broadcast((HD, ACW)),
                            op0=OP.mult, op1=OP.add,
                            accum_out=ya_part[:, c:c + 1])
                        # unscramble (wx, ty, tx) -> spatial rows 7c..7c+6
                        nc.vector.tensor_copy(
                            yav[:, c * WS:(c + 1) * WS].rearrange(
                                "p ty (wx tx) -> p wx ty tx", tx=WS),
                            yawm[:, cs].rearrange(
                                "p (wx ty tx) -> p wx ty tx", ty=WS, tx=WS))

                if KPHASE == "attn":
                    bail(img, X)
                    continue

                # ---------------- mamba branch ----------------
                xzp = gp.tile([DI, (H + 2) * (W + 2)], BF16, tag="xzp")
                nc.vector.memset(xzp, 0.0)
                xzpv = xzp.rearrange("p (h w) -> p h w", h=H + 2)
                u = gp.tile([DI, L], BF16, tag="u")
                mctx = ExitStack()
                with mctx:
                    pxz = ExitStack()
                    with pxz:
                        pmm = pxz.enter_context(
                            tc.tile_pool(name=f"pxz{img}", bufs=2, space="PSUM"))
                        for c in range(NCH):
                            cs = slice(c * CW, (c + 1) * CW)
                            ps = pmm.tile([DI, CW], FP32, tag="xz")
                            nc.tensor.matmul(ps, m_in_wT[HD:DIM],
                                             xm[HD:DIM, cs],
                                             start=True, stop=True)
                            nc.vector.tensor_copy(
                                xzpv[:, 1 + c * RCH:1 + (c + 1) * RCH, 1:W + 1],
                                ps.rearrange("p (a b) -> p a b", a=RCH))
                    acc0 = gp.tile([DI, L], BF16, tag="acc0")
                    acc1 = gp.tile([DI, L], BF16, tag="acc1")
                    src = [acc0, acc1]
                    idx = 0
                    for dy in range(3):
                        for dx in range(3):
                            shv = xzpv[:, dy:dy + H, dx:dx + W]
                            wcol = convw[:, 3 * dy + dx:3 * dy + dx + 1]
                            if dy == 0 and dx == 0:
                                nc.vector.tensor_scalar_mul(
                                    acc0.rearrange("p (h w) -> p h w", h=H),
                                    shv, wcol)
                            else:
                                nc.vector.scalar_tensor_tensor(
                                    src[1 - idx].rearrange(
                                        "p (h w) -> p h w", h=H),
                                    shv, wcol,
                                    src[idx].rearrange("p (h w) -> p h w", h=H),
                                    op0=OP.mult, op1=OP.add)
                                idx = 1 - idx
                    sgt = gp.tile([DI, L], BF16, tag="sgt")
                    nc.scalar.activation(sgt, src[idx], AF.Sigmoid,
                                         bias=m_conv_b)
                    nc.vector.scalar_tensor_tensor(u, src[idx], m_conv_b, sgt,
                                                   op0=OP.add, op1=OP.mult)

                    # scan-phase setup: PSUM y accumulator + D skip term
                    yctx = ExitStack()
                    with yctx:
                        py = yctx.enter_context(
                            tc.tile_pool(name=f"py{img}", bufs=1, space="PSUM"))
                        pkm = yctx.enter_context(
                            tc.tile_pool(name=f"pkm{img}", bufs=1, space="PSUM"))
                        spool = yctx.enter_context(
                            tc.tile_pool(name=f"scan{img}", bufs=2))
                        bcpool = yctx.enter_context(
                            tc.tile_pool(name=f"bc{img}", bufs=3))
                        y_ps = py.tile([DI, NCH, 512], FP32)
                        if KPHASE != "pre":
                            tmpd = gp.tile([DI, L], BF16, tag="xm")
                            nc.vector.tensor_scalar_mul(tmpd, u, Dsum)
                            for c in range(NCH):
                                nc.tensor.matmul(
                                    y_ps[:, c, 0:CW], ident,
                                    tmpd[:, c * CW:(c + 1) * CW],
                                    start=True, stop=False,
                                    skip_group_check=True)

                        for k in range(K):
                            # per-direction projections
                            xdbl = gp.tile([96, L], BF16, tag="xdbl")
                            for c in range(NCH):
                                cs = slice(c * CW, (c + 1) * CW)
                                ps = pkm.tile([96, CW], FP32, tag="mm")
                                nc.tensor.matmul(ps, xprojP[:, k],
                                                 u[:, cs], start=True,
                                                 stop=True)
                                nc.vector.tensor_copy(xdbl[:, cs], ps)
                            delta = gp.tile([DI, L], BF16,
                                            tag="delta0" if k % 2 == 0 else "delta1")
                            for c in range(NCH):
                                cs = slice(c * CW, (c + 1) * CW)
                                ps = pkm.tile([DI, CW], FP32, tag="mm")
                                nc.tensor.matmul(
                                    ps, dtwT[:, k * DI:(k + 1) * DI],
                                    xdbl[0:DTR, cs], start=True, stop=True)
                                # softplus(x) = ln(1 + exp(x))
                                nc.scalar.activation(ps, ps, AF.Exp,
                                                     bias=dtb[:, k:k + 1])
                                nc.scalar.activation(delta[:, cs], ps, AF.Ln,
                                                     bias=onescolf)
                            du = gp.tile([DI, L], BF16,
                                         tag="du0" if k % 2 == 0 else "du1")
                            nc.vector.tensor_mul(du, delta, u)
                            Bst_t = gp.tile([48, L], BF16, tag="acc0")
                            Cst_t = gp.tile([80, L], BF16, tag="acc1")
                            Bst = Bst_t[32:32 + DST]
                            Cst = Cst_t[64:64 + DST]
                            nc.gpsimd.tensor_copy(
                                m2(Bst, k), kv(xdbl, k)[32:32 + DST])
                            nc.gpsimd.tensor_copy(
                                m2(Cst, k), kv(xdbl, k)[64:64 + DST])
                            nc.sync.dma_start(stageB[img, k], Bst)
                            nc.sync.dma_start(stageC[img, k], Cst)

                            for n in range(DST if KPHASE != "pre" else 0):
                                a_n = spool.tile([DI, L], BF16, tag="a")
                                nc.scalar.activation(
                                    m2(a_n, k), kv(delta, k), AF.Exp,
                                    bias=zc,
                                    scale=Aneg[:, k * DST + n:k * DST + n + 1])
                                Bb = bcpool.tile([DI, L], BF16, tag="bc")
                                nc.sync.dma_start(Bb, brow(stageB[img, k, n], DI))
                                b_n = spool.tile([DI, L], BF16, tag="b")
                                nc.vector.tensor_mul(m2(b_n, k), kv(du, k), Bb
                                                     if k in (0, 2)
                                                     else m2(Bb, k))
                                h_n = spool.tile([DI, L], BF16, tag="h")
                                nc.vector.tensor_tensor_scan(
                                    h_n, a_n, b_n, 0.0,
                                    op0=OP.mult, op1=OP.add)
                                Cb = bcpool.tile([DI, L], BF16, tag="bc")
                                nc.sync.dma_start(Cb, brow(stageC[img, k, n], DI))
                                tmp = spool.tile([DI, L], BF16, tag="b")
                                # write spatially (scan-order inputs)
                                nc.vector.tensor_mul(kv(tmp, k), m2(h_n, k),
                                                     m2(Cb, k))
                                last = (k == K - 1 and n == DST - 1)
                                for c in range(NCH):
                                    nc.tensor.matmul(
                                        y_ps[:, c, 0:CW], ident,
                                        tmp[:, c * CW:(c + 1) * CW],
                                        start=False, stop=last,
                                        skip_group_check=True)
                        ymg = gp.tile([DI, L], BF16, tag="xn")  # reuse xn
                        ysg = gp.tile([DI, L], BF16, tag="sgt")
                        for c in range(NCH if KPHASE != "pre" else 0):
                            cs = slice(c * CW, (c + 1) * CW)
                            nc.scalar.activation(ysg[:, cs], y_ps[:, c, 0:CW],
                                                 AF.Sigmoid, bias=zc,
                                                 scale=1.702)
                            nc.vector.tensor_mul(ymg[:, cs], y_ps[:, c, 0:CW],
                                                 ysg[:, cs])

                if KPHASE in ("pre", "scan"):
                    bail(img, X)
                    continue

                ym = gp.tile([DIM, L], BF16, tag="xdbl")   # reuse xdbl slot
                poctx = ExitStack()
                with poctx:
                    pp = poctx.enter_context(
                        tc.tile_pool(name=f"pom{img}", bufs=2, space="PSUM"))
                    for c in range(NCH):
                        cs = slice(c * CW, (c + 1) * CW)
                        ps = pp.tile([DIM, CW], FP32, tag="mm")
                        nc.tensor.matmul(ps[HD:DIM], m_out_wT, ymg[:, cs],
                                         start=True, stop=True)
                        nc.vector.tensor_copy(ym[HD:DIM, cs], ps[HD:DIM])

                # ---------------- SE + residual ----------------
                sectx = ExitStack()
                with sectx:
                    pse = sectx.enter_context(
                        tc.tile_pool(name=f"pse{img}", bufs=1, space="PSUM"))
                    pooled = gp.tile([DIM, 1], FP32, tag="pool")
                    nc.vector.reduce_sum(pooled[0:HD], ya_part, AX.X)
                    nc.vector.reduce_sum(pooled[HD:DIM], ym[HD:DIM], AX.X)
                    nc.vector.tensor_scalar_mul(pooled, pooled, 1.0 / L)
                    p_bf = gp.tile([DIM, 1], BF16, tag="pbf")
                    nc.vector.tensor_copy(p_bf, pooled)
                    s1_ps = pse.tile([DIM // 4, 1], FP32, tag="s1")
                    nc.tensor.matmul(s1_ps, se_w1T, p_bf, start=True, stop=True)
                    s1 = gp.tile([DIM // 4, 1], BF16, tag="s1")
                    nc.scalar.activation(s1, s1_ps, AF.Relu, bias=se_b1)
                    s2_ps = pse.tile([DIM, 1], FP32, tag="s2")
                    nc.tensor.matmul(s2_ps, se_w2T, s1, start=True, stop=True)
                    s_se = gp.tile([DIM, 1], FP32, tag="sse")
                    nc.scalar.activation(s_se, s2_ps, AF.Sigmoid, bias=se_b2)
                    sg = gp.tile([DIM, 1], FP32, tag="sg")
                    nc.vector.tensor_mul(sg, s_se, gamma1)

                xp = gp.tile([DIM, L], BF16, tag="xsq")         # reuse xsq
                nc.vector.scalar_tensor_tensor(xp[0:HD], ya, sg[0:HD], X[0:HD],
                                               op0=OP.mult, op1=OP.add)
                nc.vector.scalar_tensor_tensor(xp[HD:DIM], ym[HD:DIM],
                                               sg[HD:DIM], X[HD:DIM],
                                               op0=OP.mult, op1=OP.add)

                # ---------------- MLP ----------------
                xpsq = gp.tile([DIM, L], BF16, tag="u")         # reuse u
                nc.scalar.activation(xpsq, xp, AF.Square, bias=zc)
                xh = gp.tile([DIM, L], BF16, tag="v")           # reuse v
                mlpctx = ExitStack()
                with mlpctx:
                    pnP = mlpctx.enter_context(
                        tc.tile_pool(name=f"pnP{img}", bufs=1, space="PSUM"))
                    ph1 = mlpctx.enter_context(
                        tc.tile_pool(name=f"ph1{img}", bufs=1, space="PSUM"))
                    ph2 = mlpctx.enter_context(
                        tc.tile_pool(name=f"ph2{img}", bufs=2, space="PSUM"))
                    opool = mlpctx.enter_context(
                        tc.tile_pool(name=f"out{img}", bufs=2))
                    for c in range(NCH):
                        cs = slice(c * CW, (c + 1) * CW)
                        ps = pnP.tile([1, CW], FP32, tag="rsum")
                        nc.tensor.matmul(ps, onescol, xpsq[:, cs],
                                         start=True, stop=True)
                        rv = rvp.tile([1, CW], FP32, tag="rv")
                        nc.scalar.activation(rv, ps, AF.Sqrt, bias=epsc[0:1],
                                             scale=1.0 / DIM)
                        nc.vector.reciprocal(rv, rv)
                        bc = pnP.tile([DIM, CW], FP32, tag="bc")
                        nc.tensor.matmul(bc, onesrow, rv, start=True, stop=True)
                        nc.vector.tensor_mul(xh[:, cs], xp[:, cs], bc)
                    for c in range(NCH):
                        cs = slice(c * CW, (c + 1) * CW)
                        h1_ps = ph1.tile([DIM, 4, 512], FP32, tag="h1")
                        for j in range(4):
                            nc.tensor.matmul(
                                h1_ps[:, j, 0:CW],
                                mlp_w1T[:, j * DIM:(j + 1) * DIM],
                                xh[:, cs], start=True, stop=True)
                        h1g = opool.tile([DIM, 4, CW], BF16, tag="h1g")
                        hsg = opool.tile([DIM, 4, CW], BF16, tag="hsg")
                        for j in range(4):
                            nc.scalar.activation(hsg[:, j], h1_ps[:, j, 0:CW],
                                                 AF.Sigmoid, scale=1.702,
                                                 bias=mlp_b1s[:, j:j + 1])
                            nc.vector.scalar_tensor_tensor(
                                h1g[:, j], h1_ps[:, j, 0:CW],
                                mlp_b1[:, j:j + 1], hsg[:, j],
                                op0=OP.add, op1=OP.mult)
                        h2_ps = ph2.tile([DIM, CW], FP32, tag="h2")
                        for j in range(4):
                            nc.tensor.matmul(h2_ps, mlp_w2T[:, j],
                                             h1g[:, j], start=(j == 0),
                                             stop=(j == 3))
                        hb = opool.tile([DIM, CW], BF16, tag="hb")
                        nc.vector.tensor_scalar_add(hb, h2_ps, mlp_b2)
                        outt = opool.tile([DIM, CW], FP32, tag="outt")
                        nc.vector.scalar_tensor_tensor(
                            outt, hb, gamma2, xp[:, cs],
                            op0=OP.mult, op1=OP.add)
                        nc.sync.dma_start(odram[img][:, cs], outt)


_BUILD_CACHE = {}


def _build():
    if "nc" in _BUILD_CACHE:
        return _BUILD_CACHE["nc"]
    import concourse.bacc as bacc
    import concourse.tile as tile
    from concourse import mybir
    nc = bacc.Bacc("TRN2", target_bir_lowering=False, debug=False,
                   num_devices=N_CORES)
    ins = {}
    for name, shape in _INPUT_SPECS:
        ins[name] = nc.dram_tensor(name, list(shape), mybir.dt.float32,
                                   kind="ExternalInput").ap()
    out = nc.dram_tensor("out", [B_SH, DIM, H, W], mybir.dt.float32,
                         kind="ExternalOutput").ap()
    with tile.TileContext(nc) as tc:
        emit(tc, out, ins)
    nc.compile()
    _BUILD_CACHE["nc"] = nc
    return nc


def kernel(**inputs):
    from concourse.bass_utils import run_bass_kernel_spmd
    inputs = {k: np.ascontiguousarray(np.asarray(v, dtype=np.float32))
              for k, v in inputs.items()}
    x = inputs.pop("x")
    nc = _build()
    in_maps = []
    for i in range(N_CORES):
        m = dict(inputs)
        m["x"] = np.ascontiguousarray(x[i * B_SH:(i + 1) * B_SH])
        in_maps.append(m)
    res = run_bass_kernel_spmd(nc, in_maps, core_ids=list(range(N_CORES)))
    outs = [res.results[i]["out"] for i in range(N_CORES)]
    return np.concatenate(outs, axis=0).reshape(B, DIM, H, W).astype(np.float32)
